# revision 1
# baseline (speedup 1.0000x reference)
"""KMeans assignment kernel for Trainium2 (8 NeuronCores, SPMD).

argmin_k ||f_n - c_k||^2  ==  argmax_k (2*f.c_k - ||c_k||^2)   (x_sq drop is
order-preserving).  Cross products run on the PE array with a 3-pass f16
hi/lo split at 1 cycle/row:
    f@c ~= fh16@c_hi16 + fh16@c_lo16 + fl16@c_hi16
where fh16 = f16(f), fl16 = f16(f - fh16) (likewise for the centroids).
Total abs err ~2e-4 on the 2*cross scale — fp32-grade, 0 argmin flips vs the
fp32 reference on the benchmark input.

All matmul stationaries are the (transposed) f16 feature tiles, so each
weight tile is loaded with ONE explicit 2-byte Ldweights shared by the 4 (hi)
or 2 (lo) 512-col streams that use it (_dedup_ldweights removes the
legalizer's redundant reloads; walrus runs with --enable-ldw-opt=false so
this matters).  fh16 transposes on the PE as f16 (1 cyc/row, half the cost of
the fp32 transpose), fl16 on the DMA xbar (off the PE entirely).  Per
row-tile the PE does 24 x 512-col streams + 4 x 128-row f16 transposes + 8
weight loads ~= 12.8k cycles.  The -|c|^2 bias is added by a DVE
tensor-tensor op (PSUM + broadcast row), then row-wise argmax via DVE
max/max_index.

Sharding: features split over N across 8 cores (data parallel); centroids
replicated; no cross-core communication.
"""
import sys

sys.path.insert(0, "/opt/trn_rl_repo")

import os
import numpy as np
from contextlib import ExitStack, nullcontext

import concourse.bacc as bacc
import concourse.mybir as mybir
from concourse import tile
from concourse.bass_utils import run_bass_kernel_spmd
from concourse.masks import make_identity

N, D, K = 131072, 512, 1024
N_CORES = 8
N_PER_CORE = N // N_CORES          # 16384
N_TILES = N_PER_CORE // 128        # 128 row-tiles per core
ND = D // 128                      # 4 contraction tiles
F32 = mybir.dt.float32
F32R = mybir.dt.float32r
F16 = mybir.dt.float16
U32 = mybir.dt.uint32

_cached = {}

# shipped configuration (see build_bass options).  hw-validated: 0 argmin
# flips, ~792-838us (host-dependent) vs 868-927us for the v1 baseline.
# coalesce=True passes sim but was never hw-validated -- do not enable.
SHIP_KW = {"v2": True, "moving_hi": "f16", "tp_mode": "hybrid", "act_split": 4}


def _dedup_ldweights(nc):
    """Remove InstLdweights that reload weights already resident in the PE.

    The tile legalizer emits one Ldweights per 2-byte matmul; consecutive
    matmuls sharing a stationary operand reload it redundantly.  Runs after
    TileContext exit, before nc.finalize().  Any waits on a removed ld are
    moved to the next instruction.
    """
    removed = 0
    for fn in nc.m.functions:
        for b in fn.blocks:
            insts = b.instructions
            out = []
            loaded_key = None
            pending_waits = []
            for inst in insts:
                nm = type(inst).__name__
                if nm == "InstLdweights":
                    ap = inst.ins[0]
                    key = (str(ap.memref), ap.offset, str(ap.ap), str(ap.dtype))
                    if key == loaded_key:
                        si = inst.sync_info
                        if si is not None and len(si.on_wait) > 0:
                            pending_waits.extend(si.on_wait)
                        if si is not None and len(si.on_update) > 0:
                            out.append(inst)  # has side effects: keep
                            continue
                        removed += 1
                        continue
                    loaded_key = key
                elif nm in ("InstMatmult", "InstMatmultMx"):
                    wdt = inst.ins[1].dtype
                    if mybir.dt.size(wdt) == 4:
                        loaded_key = None  # self-loading matmul clobbers PE
                if pending_waits:
                    si = inst.sync_info
                    waits = list(si.on_wait) if si else []
                    ups = list(si.on_update) if si else []
                    inst.sync_info = mybir.SyncInfo(
                        on_wait=waits + pending_waits, on_update=ups)
                    pending_waits = []
                out.append(inst)
            if removed:
                b.instructions = out
    return removed


def _coalesce_pe_incs(nc):
    """Coalesce per-instruction PE semaphore increments.

    Every PE matmul/transpose increments the PE event semaphore by 1, but
    consumers only wait on a few specific cumulative counts (after the last
    transpose of a tile, after the last matmul into a PSUM buffer).  Strip
    the increments nobody can observe and fold them into the next observed
    point, keeping every waited-on threshold exact.  Runs pre-finalize.
    """
    import collections
    changed = 0
    for fn in nc.m.functions:
        # thresholds collected across ALL blocks (waits may live elsewhere)
        thresholds = collections.defaultdict(set)
        for bb in fn.blocks:
            for inst in bb.instructions:
                si = inst.sync_info
                if si is None:
                    continue
                for w in si.on_wait:
                    if w.sync_type == "semaphore" and w.wait_mode == "sem-ge-imm":
                        thresholds[w.id].add(w.wait_value)
        # sems safe to coalesce: incremented ONLY by PE sem-inc updates and
        # never written/subtracted/reg-waited anywhere
        inc_by_pe = set()
        unsafe = set()
        for bb in fn.blocks:
            for inst in bb.instructions:
                si = inst.sync_info
                if si is None:
                    continue
                is_pe = str(inst.engine).endswith("PE")
                for u in si.on_update:
                    if u.sync_type != "semaphore":
                        continue
                    if u.update_mode == "sem-inc" and is_pe:
                        inc_by_pe.add(u.id)
                    else:
                        unsafe.add(u.id)
                for w in si.on_wait:
                    if w.sync_type == "semaphore" and w.wait_mode != "sem-ge-imm":
                        unsafe.add(w.id)
        for b in fn.blocks:
            insts = b.instructions
            pe_sems = inc_by_pe - unsafe
            for sem in pe_sems:
                th = thresholds[sem]
                # walk PE instrs carrying incs on this sem
                carriers = []
                count = 0
                for inst in insts:
                    si = inst.sync_info
                    if si is None or not str(inst.engine).endswith("PE"):
                        continue
                    ups = [u for u in si.on_update
                           if u.sync_type == "semaphore" and u.id == sem
                           and u.update_mode == "sem-inc"]
                    if ups:
                        count += sum(u.update_value for u in ups)
                        carriers.append((inst, ups, count))
                pending = 0
                for i, (inst, ups, cum) in enumerate(carriers):
                    inc_here = sum(u.update_value for u in ups)
                    is_needed = (cum in th) or (i == len(carriers) - 1)
                    si = inst.sync_info
                    if is_needed:
                        if pending:
                            # fold the stripped increments into this one
                            keep = [u for u in si.on_update
                                    if not (u.sync_type == "semaphore"
                                            and u.id == sem
                                            and u.update_mode == "sem-inc")]
                            keep.append(mybir.SyncUpdate(
                                sync_type="semaphore", id=sem,
                                ant_name=ups[0].ant_name,
                                update_mode="sem-add-imm",
                                update_value=inc_here + pending))
                            inst.sync_info = mybir.SyncInfo(
                                on_wait=list(si.on_wait), on_update=keep)
                            changed += 1
                        pending = 0
                    else:
                        keep = [u for u in si.on_update
                                if not (u.sync_type == "semaphore"
                                        and u.id == sem
                                        and u.update_mode == "sem-inc")]
                        inst.sync_info = mybir.SyncInfo(
                            on_wait=list(si.on_wait), on_update=keep)
                        pending += inc_here
                        changed += 1
    return changed


def build_bass_v2(n_tiles: int = N_TILES, repeat: int = 1,
                  psum_bufs: int = 3, work_bufs: int = 3,
                  moving_hi: str = "f32r", dedup: bool = True,
                  do_argmax: bool = True, dma_tp: bool = True,
                  bias_mode: str = "dve", skip_mm: bool = False,
                  tp_mode: str = None, psA_bufs: int = 2,
                  cvt_engine: str = "scalar", argmax_mode: str = "maxidx",
                  act_split: int = 1, probe_mm: bool = False,
                  coalesce: bool = False):
    """v2: all matmul stationaries are f16 feature tiles (so every weight
    load is an explicit 2-byte Ldweights, deduplicated across the matmuls
    that share it), centroids stream as f32r hi + f16 lo, and the feature
    transposes run on the DMA xbar (2-byte tile transpose) instead of the
    PE.  Per row-tile the PE does only 24 x 512-col streams + 8 weight
    loads.

        f@c ~= fh16 @ c_hi  +  fh16 @ c_lo16  +  fl16 @ c_hi
        fh16 = f16(f);  fl16 = f16(f - fh16);  c_hi = f32r(c);
        c_lo16 = f16(c - c_hi)            (err ~2^-23 rel of the f@c scale)
    """
    n_rows = n_tiles * 128
    nc = bacc.Bacc()
    feat = nc.declare_dram_parameter("features", [n_rows, D], F32, isOutput=False)
    cent = nc.declare_dram_parameter("centroids", [D, K], F32, isOutput=False)
    ncsq = nc.declare_dram_parameter("ncsq", [1, K], F32, isOutput=False)
    out = nc.declare_dram_parameter("out", [n_rows, 1], F32, isOutput=True)

    CH = F32R if moving_hi == "f32r" else F16

    with tile.TileContext(nc) as tc, ExitStack() as ctx:
        const = ctx.enter_context(tc.tile_pool(name="const", bufs=1))
        work = ctx.enter_context(tc.tile_pool(name="work", bufs=work_bufs))
        tps = ctx.enter_context(tc.tile_pool(name="tps", bufs=work_bufs))
        red = ctx.enter_context(tc.tile_pool(name="red", bufs=4))
        psB = ctx.enter_context(tc.tile_pool(name="psB", bufs=psum_bufs, space="PSUM"))

        if tp_mode is None:
            tp_mode = "dma" if dma_tp else "pe"
        if tp_mode == "pe":
            ident = const.tile([128, 128], F32)
            make_identity(nc, ident[:])
            psA = ctx.enter_context(tc.tile_pool(name="psA", bufs=psA_bufs, space="PSUM"))
        elif tp_mode == "hybrid":
            identf = const.tile([128, 128], F32)
            make_identity(nc, identf[:])
            ident = const.tile([128, 128], F16)
            nc.vector.tensor_copy(out=ident[:], in_=identf[:])
            psA = ctx.enter_context(tc.tile_pool(name="psA", bufs=psA_bufs, space="PSUM"))

        # centroids resident in SBUF, layout [128, ND*K]: hi (f32r or f16) + f16 lo
        ctile = const.tile([128, ND * K], F32)
        nc.sync.dma_start(
            out=ctile[:].rearrange("p (a k) -> p a k", a=ND),
            in_=cent[:].rearrange("(a p) k -> p a k", p=128),
        )
        c_hi = const.tile([128, ND * K], CH)
        c_lo16 = const.tile([128, ND * K], F16)
        nc.vector.tensor_copy(out=c_hi[:], in_=ctile[:])
        chi_in = c_hi[:].bitcast(F32) if moving_hi == "f32r" else c_hi[:]
        nc.vector.tensor_tensor(out=c_lo16[:], in0=ctile[:], in1=chi_in,
                                op=mybir.AluOpType.subtract)

        ncsq_t = const.tile([1, K], F32)
        nc.sync.dma_start(out=ncsq_t[:], in_=ncsq[:])
        ncsq_b = const.tile([128, K], F32)
        nc.gpsimd.partition_broadcast(ncsq_b[:], ncsq_t[:])

        idx8 = None
        if do_argmax:
            idx8 = const.tile([128, n_tiles * 8], U32, tag="idx8")
        fbuf = const.tile([128, n_tiles], F32)

        if probe_mm:
            # constant transposed weights: isolates the PE ld/mm stream
            cfh = const.tile([128, ND * 128], F16)
            cfl = const.tile([128, ND * 128], F16)
            nc.vector.memset(cfh[:], 0.25)
            nc.vector.memset(cfl[:], 0.001)

        loop_ctx = tc.For_i(0, repeat, 1) if repeat > 1 else nullcontext()
        with loop_ctx:
            for rt in range(n_tiles):
                if probe_mm:
                    mp = psB.tile([128, K], F32, tag="mp")
                    for d in range(ND):
                        for mv_t in (c_hi, c_lo16):
                            for kh in range(2):
                                nc.tensor.matmul(
                                    mp[:, kh * 512:(kh + 1) * 512],
                                    lhsT=cfh[:, d * 128:(d + 1) * 128],
                                    rhs=mv_t[:, d * K + kh * 512:d * K + (kh + 1) * 512],
                                    start=(d == 0 and mv_t is c_hi), stop=False)
                    for d in range(ND):
                        for kh in range(2):
                            nc.tensor.matmul(
                                mp[:, kh * 512:(kh + 1) * 512],
                                lhsT=cfl[:, d * 128:(d + 1) * 128],
                                rhs=c_hi[:, d * K + kh * 512:d * K + (kh + 1) * 512],
                                start=False, stop=(d == ND - 1))
                    continue
                ftile = work.tile([128, D], F32, tag="ftile")
                nc.sync.dma_start(out=ftile[:], in_=feat[rt * 128:(rt + 1) * 128, :])

                fh16T = tps.tile([128, ND * 128], F16, tag="fh16T")
                fl16T = tps.tile([128, ND * 128], F16, tag="fl16T")
                if tp_mode == "dma":
                    # f16 hi/lo split in row-major layout, then DMA-xbar
                    # transpose -> [d-part, a, rows]
                    fh16 = work.tile([128, D], F16, tag="fh16")
                    fl16 = work.tile([128, D], F16, tag="fl16")
                    nc.scalar.copy(out=fh16[:], in_=ftile[:])
                    nc.vector.tensor_tensor(out=fl16[:], in0=ftile[:], in1=fh16[:],
                                            op=mybir.AluOpType.subtract)
                    nc.sync.dma_start_transpose(
                        fh16T[:].rearrange("p (a r) -> p a r", a=ND), fh16[:])
                    nc.sync.dma_start_transpose(
                        fl16T[:].rearrange("p (a r) -> p a r", a=ND), fl16[:])
                elif tp_mode == "hybrid":
                    # hi: f16 transpose on PE (1 cyc/row); lo: DMA xbar
                    fh16 = work.tile([128, D], F16, tag="fh16")
                    fl16 = work.tile([128, D], F16, tag="fl16")
                    if cvt_engine == "pool":
                        nc.gpsimd.tensor_copy(out=fh16[:], in_=ftile[:])
                    else:
                        nc.scalar.copy(out=fh16[:], in_=ftile[:])
                    nc.vector.tensor_tensor(out=fl16[:], in0=ftile[:], in1=fh16[:],
                                            op=mybir.AluOpType.subtract)
                    tp16 = psA.tile([128, ND * 128], F16, tag="tp16")
                    for d in range(ND):
                        nc.tensor.transpose(tp16[:, d * 128:(d + 1) * 128],
                                            fh16[:, d * 128:(d + 1) * 128],
                                            ident[:])
                    cw = ND * 128 // act_split
                    for s in range(act_split):
                        nc.scalar.copy(out=fh16T[:, s * cw:(s + 1) * cw],
                                       in_=tp16[:, s * cw:(s + 1) * cw])
                    nc.sync.dma_start_transpose(
                        fl16T[:].rearrange("p (a r) -> p a r", a=ND), fl16[:])
                else:
                    # fp32 transpose on the PE, split hi/lo afterwards
                    tp = psA.tile([128, ND * 128], F32, tag="tp")
                    for d in range(ND):
                        nc.tensor.transpose(tp[:, d * 128:(d + 1) * 128],
                                            ftile[:, d * 128:(d + 1) * 128],
                                            ident[:])
                    nc.scalar.copy(out=fh16T[:], in_=tp[:])
                    nc.vector.tensor_tensor(out=fl16T[:], in0=tp[:], in1=fh16T[:],
                                            op=mybir.AluOpType.subtract)

                if skip_mm:
                    continue
                # PSUM [128, K]: 24 streams, stationaries grouped per ld
                mp = psB.tile([128, K], F32, tag="mp")
                preload = bias_mode == "preload" and do_argmax
                if preload:
                    # bias lands in PSUM first; matmuls accumulate on top
                    nc.scalar.copy(out=mp[:], in_=ncsq_b[:])
                for d in range(ND):           # fh16T[d]: 4 streams per load
                    for mv_t, mv_name in ((c_hi, "hi"), (c_lo16, "lo")):
                        for kh in range(2):
                            nc.tensor.matmul(
                                mp[:, kh * 512:(kh + 1) * 512],
                                lhsT=fh16T[:, d * 128:(d + 1) * 128],
                                rhs=mv_t[:, d * K + kh * 512:d * K + (kh + 1) * 512],
                                start=(d == 0 and mv_name == "hi" and not preload),
                                stop=False, skip_group_check=preload)
                for d in range(ND):           # fl16T[d]: 2 streams per load
                    for kh in range(2):
                        nc.tensor.matmul(
                            mp[:, kh * 512:(kh + 1) * 512],
                            lhsT=fl16T[:, d * 128:(d + 1) * 128],
                            rhs=c_hi[:, d * K + kh * 512:d * K + (kh + 1) * 512],
                            start=False,
                            stop=(d == ND - 1), skip_group_check=preload)

                if not do_argmax:
                    continue
                mv = red.tile([128, 8], F32, tag="mv")
                if preload:
                    nc.vector.max(mv[:], mp[:])
                    nc.vector.max_index(idx8[:, rt * 8:(rt + 1) * 8], mv[:], mp[:])
                elif argmax_mode == "fused":
                    # one DVE pass: m_s = mp + bias, mx1 = rowmax(m_s)
                    m_s = work.tile([128, K], F32, tag="m_s")
                    mx1 = red.tile([128, 1], F32, tag="mx1")
                    nc.vector.tensor_tensor_reduce(
                        out=m_s[:], in0=mp[:], in1=ncsq_b[:], scale=1.0,
                        scalar=-3.0e38, op0=mybir.AluOpType.add,
                        op1=mybir.AluOpType.max, accum_out=mx1[:])
                    nc.vector.max_index(idx8[:, rt * 8:(rt + 1) * 8],
                                        mx1[:].broadcast_to([128, 8]), m_s[:])
                else:
                    m_s = work.tile([128, K], F32, tag="m_s")
                    bias_eng = nc.gpsimd if bias_mode == "pool" else nc.vector
                    bias_eng.tensor_tensor(out=m_s[:], in0=mp[:], in1=ncsq_b[:],
                                           op=mybir.AluOpType.add)
                    nc.vector.max(mv[:], m_s[:])
                    nc.vector.max_index(idx8[:, rt * 8:(rt + 1) * 8], mv[:], m_s[:])

        if do_argmax:
            nc.vector.tensor_copy(out=fbuf[:], in_=idx8[:, 0:n_tiles * 8:8])
        else:
            nc.vector.memset(fbuf[:], 0.0)
        nc.sync.dma_start(out=out[:, 0].rearrange("(t p) -> p t", p=128),
                          in_=fbuf[:])

    if dedup:
        n = _dedup_ldweights(nc)
        if os.environ.get("KM_DEBUG"):
            print(f"dedup_ldweights removed {n}")
    if coalesce:
        # compile() regenerates event semaphores assuming unit increments,
        # so coalesce after it, then run the remaining finalize steps
        nc.compile()
        n = _coalesce_pe_incs(nc)
        if os.environ.get("KM_DEBUG"):
            print(f"coalesce_pe_incs changed {n}")
        super(bacc.Bacc, nc).finalize()
    else:
        nc.finalize()
    return nc


def build_bass(n_tiles: int = N_TILES, repeat: int = 1,
               bias_on_dve: bool = False, hilo_engine: str = "vector",
               n_passes: int = 3, do_argmax: bool = True,
               corr_f16: bool = False, all_f16: bool = False,
               kh_inner: bool = False, psum_bufs: int = 2,
               wide_mm: bool = False, v2: bool = False, **v2_kw):
    if v2:
        if "psB_bufs" in v2_kw:
            v2_kw["psum_bufs"] = v2_kw.pop("psB_bufs")
        return build_bass_v2(n_tiles=n_tiles, repeat=repeat,
                             do_argmax=do_argmax, **v2_kw)
    n_rows = n_tiles * 128
    nc = bacc.Bacc()
    feat = nc.declare_dram_parameter("features", [n_rows, D], F32, isOutput=False)
    cent = nc.declare_dram_parameter("centroids", [D, K], F32, isOutput=False)
    ncsq = nc.declare_dram_parameter("ncsq", [1, K], F32, isOutput=False)
    out = nc.declare_dram_parameter("out", [n_rows, 1], F32, isOutput=True)

    with tile.TileContext(nc) as tc, ExitStack() as ctx:
        const = ctx.enter_context(tc.tile_pool(name="const", bufs=1))
        work = ctx.enter_context(tc.tile_pool(name="work", bufs=3))
        red = ctx.enter_context(tc.tile_pool(name="red", bufs=4))
        psA = ctx.enter_context(tc.tile_pool(name="psA", bufs=2, space="PSUM"))
        psB = ctx.enter_context(tc.tile_pool(name="psB", bufs=psum_bufs, space="PSUM"))

        ident = const.tile([128, 128], F32)
        make_identity(nc, ident[:])

        # centroids resident in SBUF, split hi/lo f32r; layout [128, ND*K]
        ctile = const.tile([128, ND * K], F32)
        nc.sync.dma_start(
            out=ctile[:].rearrange("p (a k) -> p a k", a=ND),
            in_=cent[:].rearrange("(a p) k -> p a k", p=128),
        )
        if all_f16:
            # pure-fp16 3-way split: h+l capture ~21 mantissa bits
            c_hi = const.tile([128, ND * K], F16)
            c_lo = const.tile([128, ND * K], F16)
            nc.vector.tensor_copy(out=c_hi[:], in_=ctile[:])
            nc.vector.tensor_tensor(out=c_lo[:], in0=ctile[:], in1=c_hi[:],
                                    op=mybir.AluOpType.subtract)
        else:
            c_hi = const.tile([128, ND * K], F32R)
            c_lo = const.tile([128, ND * K], F32R)
            nc.vector.tensor_copy(out=c_hi[:], in_=ctile[:])
            nc.vector.tensor_tensor(out=c_lo[:], in0=ctile[:], in1=c_hi[:].bitcast(F32),
                                    op=mybir.AluOpType.subtract)
        if corr_f16:
            # correction operands in fp16: 2-byte weight loads, ample precision
            # (error ~2^-11 relative of a ~2^-12-relative correction term)
            c_hi16 = const.tile([128, ND * K], F16)
            c_lo16 = const.tile([128, ND * K], F16)
            nc.vector.tensor_copy(out=c_hi16[:], in_=ctile[:])
            nc.vector.tensor_copy(out=c_lo16[:], in_=c_lo[:].bitcast(F32))

        # -|c|^2 bias row, split hi/lo; plus a ones row for rank-1 matmuls
        ncsq_t = const.tile([1, K], F32)
        nc.sync.dma_start(out=ncsq_t[:], in_=ncsq[:])
        if bias_on_dve:
            ncsq_b = const.tile([128, K], F32)
            nc.gpsimd.partition_broadcast(ncsq_b[:], ncsq_t[:])
        else:
            ncsq_hi = const.tile([1, K], F32R)
            ncsq_lo = const.tile([1, K], F32R)
            nc.vector.tensor_copy(out=ncsq_hi[:], in_=ncsq_t[:])
            nc.vector.tensor_tensor(out=ncsq_lo[:], in0=ncsq_t[:],
                                    in1=ncsq_hi[:].bitcast(F32),
                                    op=mybir.AluOpType.subtract)
            ones_f = const.tile([1, 128], F32)
            nc.vector.memset(ones_f[:], 1.0)
            ones_t = const.tile([1, 128], F32R)
            nc.vector.tensor_copy(out=ones_t[:], in_=ones_f[:])

        # per-row argmax indices accumulate here ([p, t*8] layout), cast at end
        idx8 = None
        if do_argmax:
            idx8 = const.tile([128, n_tiles * 8], U32, tag="idx8")
        fbuf = const.tile([128, n_tiles], F32)

        hilo = {"vector": nc.vector, "gpsimd": nc.gpsimd}.get(hilo_engine)

        loop_ctx = tc.For_i(0, repeat, 1) if repeat > 1 else nullcontext()
        with loop_ctx:
            for rt in range(n_tiles):
                ftile = work.tile([128, D], F32, tag="ftile")
                nc.sync.dma_start(out=ftile[:], in_=feat[rt * 128:(rt + 1) * 128, :])

                # transpose features tile -> [D, rows] chunks (exact fp32)
                tp = psA.tile([128, ND * 128], F32, tag="tp")
                for d in range(ND):
                    nc.tensor.transpose(tp[:, d * 128:(d + 1) * 128],
                                        ftile[:, d * 128:(d + 1) * 128], ident[:])
                ftT = work.tile([128, D], F32, tag="ftT")
                nc.scalar.copy(out=ftT[:], in_=tp[:])

                # hi/lo split + per-pass operand prep
                if all_f16:
                    f_hi = work.tile([128, D], F16, tag="f_hi")
                    f_lo = work.tile([128, D], F16, tag="f_lo")
                    nc.scalar.copy(out=f_hi[:], in_=ftT[:])
                    nc.vector.tensor_tensor(out=f_lo[:], in0=ftT[:], in1=f_hi[:],
                                            op=mybir.AluOpType.subtract)
                    passes_all = ((f_hi, c_hi), (f_hi, c_lo), (f_lo, c_hi))
                elif corr_f16:
                    f_hi = work.tile([128, D], F32R, tag="f_hi")
                    nc.vector.tensor_copy(out=f_hi[:], in_=ftT[:])
                    f_hi16 = work.tile([128, D], F16, tag="f_hi16")
                    f_lo16 = work.tile([128, D], F16, tag="f_lo16")
                    nc.scalar.copy(out=f_hi16[:], in_=ftT[:])
                    nc.vector.tensor_tensor(out=f_lo16[:], in0=ftT[:],
                                            in1=f_hi[:].bitcast(F32),
                                            op=mybir.AluOpType.subtract)
                    passes_all = ((f_hi, c_hi), (f_hi16, c_lo16), (f_lo16, c_hi16))
                else:
                    f_hi = work.tile([128, D], F32R, tag="f_hi")
                    f_lo = work.tile([128, D], F32R, tag="f_lo")
                    if hilo is None:  # "split": hi on ScalarE, lo on GpSimd
                        nc.scalar.copy(out=f_hi[:], in_=ftT[:])
                        nc.gpsimd.tensor_tensor(out=f_lo[:], in0=ftT[:],
                                                in1=f_hi[:].bitcast(F32),
                                                op=mybir.AluOpType.subtract)
                    else:
                        hilo.tensor_copy(out=f_hi[:], in_=ftT[:])
                        hilo.tensor_tensor(out=f_lo[:], in0=ftT[:],
                                           in1=f_hi[:].bitcast(F32),
                                           op=mybir.AluOpType.subtract)
                    passes_all = ((f_hi, c_hi), (f_hi, c_lo), (f_lo, c_hi))

                # m = 2*cross [- |c|^2] accumulated in PSUM [128, K]
                mp = psB.tile([128, K], F32, tag="mp")
                passes = passes_all[:n_passes]
                if wide_mm:
                    # single matmul spans the full K (2 PSUM banks): halves
                    # the number of weight loads
                    for pi, (fa, ca) in enumerate(passes):
                        for d in range(ND):
                            is_last_main = pi == n_passes - 1 and d == ND - 1
                            nc.tensor.matmul(
                                mp[:],
                                lhsT=fa[:, d * 128:(d + 1) * 128],
                                rhs=ca[:, d * K:(d + 1) * K],
                                start=pi == 0 and d == 0,
                                stop=bias_on_dve and is_last_main)
                elif kh_inner:
                    # consecutive MM pairs share the stationary operand and
                    # alternate PSUM banks
                    for pi, (fa, ca) in enumerate(passes):
                        for d in range(ND):
                            is_last_main = pi == n_passes - 1 and d == ND - 1
                            for kh in range(2):
                                nc.tensor.matmul(
                                    mp[:, kh * 512:(kh + 1) * 512],
                                    lhsT=fa[:, d * 128:(d + 1) * 128],
                                    rhs=ca[:, d * K + kh * 512:d * K + (kh + 1) * 512],
                                    start=pi == 0 and d == 0,
                                    stop=bias_on_dve and is_last_main)
                else:
                    for kh in range(2):
                        ks = slice(kh * 512, (kh + 1) * 512)
                        mslc = mp[:, ks]
                        first = True
                        for pi, (fa, ca) in enumerate(passes):
                            for d in range(ND):
                                is_last_main = pi == n_passes - 1 and d == ND - 1
                                nc.tensor.matmul(
                                    mslc,
                                    lhsT=fa[:, d * 128:(d + 1) * 128],
                                    rhs=ca[:, d * K + kh * 512:d * K + (kh + 1) * 512],
                                    start=first,
                                    stop=bias_on_dve and is_last_main)
                                first = False
                if not bias_on_dve:
                    for kh in range(2):
                        ks = slice(kh * 512, (kh + 1) * 512)
                        nc.tensor.matmul(mp[:, ks], lhsT=ones_t[:], rhs=ncsq_hi[:, ks],
                                         start=False, stop=False)
                        nc.tensor.matmul(mp[:, ks], lhsT=ones_t[:], rhs=ncsq_lo[:, ks],
                                         start=False, stop=True)

                if not do_argmax:
                    continue
                mv = red.tile([128, 8], F32, tag="mv")
                if bias_on_dve:
                    m_s = work.tile([128, K], F32, tag="m_s")
                    nc.vector.tensor_tensor(out=m_s[:], in0=mp[:], in1=ncsq_b[:],
                                            op=mybir.AluOpType.add)
                    nc.vector.max(mv[:], m_s[:])
                    nc.vector.max_index(idx8[:, rt * 8:(rt + 1) * 8], mv[:], m_s[:])
                else:
                    nc.vector.max(mv[:], mp[:])
                    nc.vector.max_index(idx8[:, rt * 8:(rt + 1) * 8], mv[:], mp[:])

        # gather col 0 of each 8-block, cast u32 -> f32, store
        if do_argmax:
            nc.vector.tensor_copy(out=fbuf[:], in_=idx8[:, 0:n_tiles * 8:8])
        else:
            nc.vector.memset(fbuf[:], 0.0)
        nc.sync.dma_start(out=out[:, 0].rearrange("(t p) -> p t", p=128),
                          in_=fbuf[:])

    nc.finalize()
    return nc


def _get_nc():
    if "nc" not in _cached:
        _cached["nc"] = build_bass(**SHIP_KW)
    return _cached["nc"]


def kernel(features: np.ndarray, centroids: np.ndarray) -> np.ndarray:
    features = np.ascontiguousarray(np.asarray(features, dtype=np.float32))
    centroids = np.ascontiguousarray(np.asarray(centroids, dtype=np.float32))
    # PE computes f @ cent_dev; pass 2*c so PSUM holds 2*cross directly
    # (power-of-2 scaling is exact and commutes with fp32 rounding).
    cent2 = (2.0 * centroids).astype(np.float32)
    ncsq = -(centroids.astype(np.float64) ** 2).sum(0, keepdims=True).astype(np.float32)

    nc = _get_nc()
    in_maps = [
        {
            "features": features[c * N_PER_CORE:(c + 1) * N_PER_CORE],
            "centroids": cent2,
            "ncsq": ncsq,
        }
        for c in range(N_CORES)
    ]
    res = run_bass_kernel_spmd(nc, in_maps, list(range(N_CORES))).results
    out = np.concatenate([res[c]["out"] for c in range(N_CORES)], axis=0)
    return out.astype(np.float32)


def _self_test():
    rng = np.random.default_rng(0)
    f = rng.standard_normal((N, D)).astype(np.float32)
    c = rng.standard_normal((D, K)).astype(np.float32)
    out = kernel(f, c)
    x = f @ c
    ref = (-2 * x + (c * c).sum(0)).argmin(1)
    print("mismatch:", (out[:, 0] != ref).sum(), "/", N)


if __name__ == "__main__":
    _self_test()

